# revision 26
# baseline (speedup 1.0000x reference)
"""Trainium2 Bass kernel for DeepMinAttLSTM (4x minLSTM + MHSA + last-step FC).

Strategy:
  - Data-parallel over batch: 16 batches -> 8 cores x 2 batches.
  - Activations are feature-major: X^T [H=1024 (8 partition-tiles of 128),
    B*S=2048 free] in bf16; gate matmuls with W^T stationary, fp32 PSUM.
  - Gate math (per [128,512] chunk) is engine-balanced so the DVE never
    gates PSUM recycling (5.2us of matmuls vs ~4.4us of DVE per chunk):
      ACT : f = sigmoid(psF+bF), i = sigmoid(psI+bI)
      DVE : d = f+i, r = 1/d (approx), g = i*r, A = 1-g (tensor_scalar),
            B = (psH+bH)*g (stt), scan(A,B)
    h_t = A*h_{t-1} + B  ==  (f*h + i*h~)/(f+i)   (A = f/(f+i) = 1-g)
  - Chunk loop is ch-outer so each layer finishes its time-columns in the
    order the next layer consumes them (cross-layer pipelining).
  - Gate weights stream as ko-halves on two DMA queues, allocated while the
    previous layer's tiles are still live so the pool assigns fresh slots
    and the transfers run a full layer ahead (no boundary stalls).
  - Attention: output only needs the last query position, so
      scores_s = q . K_s  ==  (Wk_j^T q_j) . h4_s   (per head j)
    which removes the full K matmul; K's bias shifts all scores of a query
    equally and cancels in softmax. V is computed position-major as before.
    Softmax denominators accumulate on the PE via ones-matmuls.
  - All matmuls bf16 with fp32 accumulation.
"""

import math

import numpy as np
import ml_dtypes

BF16 = ml_dtypes.bfloat16

P = 128
H = 1024
S = 1024
B = 16
NCORES = 8
BC = B // NCORES          # batches per core
BS = BC * S               # 2048 free columns per core
KO = H // P               # 8 feature partition-tiles
NH = 8
DH = H // NH              # 128
O = 256
L = 4
QSCALE = 1.0 / math.sqrt(DH)

_CACHE = {}


def _build_nc():
    import concourse.mybir as mybir
    import concourse.tile as tile
    from concourse import bacc

    DT = mybir.dt.bfloat16
    F32 = mybir.dt.float32
    AFT = mybir.ActivationFunctionType
    OP = mybir.AluOpType

    nc = bacc.Bacc("TRN2", target_bir_lowering=False, debug=False,
                   num_devices=NCORES)

    xT = nc.dram_tensor("xT", [P, KO * BS], DT, kind="ExternalInput").ap()
    gw = nc.dram_tensor("gw", [3 * L * P, KO * H], DT, kind="ExternalInput").ap()
    gb = nc.dram_tensor("gb", [P, 3 * L * KO], F32, kind="ExternalInput").ap()
    ip = nc.dram_tensor("ip", [P, KO * 2 * H], DT, kind="ExternalInput").ap()
    ipk2 = nc.dram_tensor("ipk2", [P, NH * H], DT, kind="ExternalInput").ap()
    ipb = nc.dram_tensor("ipb", [P, 2 * KO], F32, kind="ExternalInput").ap()
    vb = nc.dram_tensor("vb", [P, NH], F32, kind="ExternalInput").ap()
    ow = nc.dram_tensor("ow", [P, KO * H], DT, kind="ExternalInput").ap()
    ob = nc.dram_tensor("ob", [P, KO], F32, kind="ExternalInput").ap()
    fcw = nc.dram_tensor("fcw", [P, KO * O], DT, kind="ExternalInput").ap()
    fcb = nc.dram_tensor("fcb", [P, O // P], F32, kind="ExternalInput").ap()
    outT = nc.dram_tensor("outT", [O, BC], F32, kind="ExternalOutput").ap()

    with tile.TileContext(nc) as tc:
        with (
            tc.tile_pool(name="constp", bufs=1) as constp,
            tc.tile_pool(name="hbuf", bufs=2) as hp,
        ):
            gb_sb = constp.tile([P, 3 * L * KO], F32)
            nc.sync.dma_start(gb_sb[:], gb[:])
            ones_col = constp.tile([P, 1], DT)
            nc.vector.memset(ones_col[:], 1.0)
            ones_row = constp.tile([1, P], F32)
            nc.vector.memset(ones_row[:], 1.0)

            X = hp.tile([P, KO * BS], DT, tag="hbuf", name="xT_sb")
            # x arrives chunk-major ([ch][ko][512]) so each chunk load is one
            # contiguous DMA; layer-0 matmuls index X accordingly
            xT_v = xT.rearrange("p m -> p m")
            X_v = X.rearrange("p m -> p m")
            nc.sync.dma_start(X_v[:, 0:512 * 4], xT_v[:, 0:512 * 4])
            nc.sync.dma_start(X_v[:, 512 * 4:512 * KO],
                              xT_v[:, 512 * 4:512 * KO])

            # in_proj weights preloaded early (pool below layer pools so the
            # DMA does not wait for layer-pool release zones)
            ip_pool = tc.tile_pool(name="ipp", bufs=1)
            ipp = ip_pool.__enter__()
            ip_sb = ipp.tile([P, KO * 2 * H], DT, name="ip_sb")

            # gate weights in a pool outside the layer scope: pool
            # release zones would otherwise delay each layer's weight DMA
            # to the end of the previous layer
            gw_pool = tc.tile_pool(name="gwp", bufs=12)
            gwp = gw_pool.__enter__()

            # V-phase PSUM banks reserved while psA is open so they
            # never alias the gate psums (no WAR stall at attention entry)
            psV_pool = tc.tile_pool(name="psV", bufs=2, space="PSUM")
            psV = psV_pool.__enter__()

            # ---------------- minLSTM layers ----------------
            with (
                tc.tile_pool(name="abp", bufs=3) as abp,
                tc.tile_pool(name="tmpp", bufs=2) as tmpp,
                tc.tile_pool(name="psA", bufs=6, space="PSUM") as psA,
            ):
                def load_gw(l):
                    # emitted while the PREVIOUS layer's tiles still have
                    # future readers, so the allocator takes fresh slots and
                    # the DMAs run ~a full layer ahead
                    res = []
                    for g in range(3):
                        lg = l * 3 + g
                        halves = []
                        for hk in range(2):
                            gw_t = gwp.tile([P, KO * H // 2], DT, tag="gw",
                                            name=f"gw_{l}_{g}_{hk}")
                            if hk == 0:
                                eng = nc.gpsimd
                            elif l == 0 and g < 2:
                                eng = nc.scalar
                            else:
                                eng = nc.sync
                            eng.dma_start(
                                gw_t[:],
                                gw[lg * P:(lg + 1) * P,
                                   hk * (KO * H // 2):(hk + 1) * (KO * H // 2)])
                            halves.append(gw_t)
                        res.append(halves)
                    return res

                gws = load_gw(0)
                for xch in range(1, 4):
                    c0 = xch * 512 * KO
                    nc.sync.dma_start(X_v[:, c0:c0 + 512 * KO],
                                      xT_v[:, c0:c0 + 512 * KO])
                for l in range(L):
                    if l == 3:
                        # overlap the 6MB in_proj load with the last layer
                        nc.sync.dma_start(ip_sb[:], ip[:])
                    h_out = hp.tile([P, KO * BS], DT, tag="hbuf", name=f"h_{l}")
                    for ch in range(4):
                        m0 = ch * 512
                        b, half = ch // 2, ch % 2
                        if ch == 1 and l + 1 < L:
                            gws_next = load_gw(l + 1)
                        for no in range(KO):
                            psF = psA.tile([P, 512], F32, tag="ps", name="psF")
                            psI = psA.tile([P, 512], F32, tag="ps", name="psI")
                            psH = psA.tile([P, 512], F32, tag="ps", name="psH")
                            for g, ps in ((0, psF), (1, psI), (2, psH)):
                                for ko in range(KO):
                                    wt = gws[g][ko // 4]
                                    kk = ko % 4
                                    if l == 0:
                                        xs = ch * 512 * KO + ko * 512
                                    else:
                                        xs = ko * BS + m0
                                    nc.tensor.matmul(
                                        ps[:],
                                        wt[:, kk * H + no * P:
                                           kk * H + (no + 1) * P],
                                        X[:, xs: xs + 512],
                                        start=(ko == 0), stop=(ko == KO - 1))
                            f_t = tmpp.tile([P, 512], DT, tag="f_t", name="f_t")
                            i_t = tmpp.tile([P, 512], DT, tag="i_t", name="i_t")
                            d_t = tmpp.tile([P, 512], F32, tag="d_t", name="d_t", bufs=1)
                            r_t = tmpp.tile([P, 512], F32, tag="r_t", name="r_t", bufs=1)
                            g_t = tmpp.tile([P, 512], DT, tag="g_t", name="g_t", bufs=1)
                            a_t = abp.tile([P, 512], DT, tag="ab", name="a_t")
                            b_t = abp.tile([P, 512], DT, tag="ab", name="b_t")
                            bF = gb_sb[:, (l * 3 + 0) * KO + no:
                                       (l * 3 + 0) * KO + no + 1]
                            bI = gb_sb[:, (l * 3 + 1) * KO + no:
                                       (l * 3 + 1) * KO + no + 1]
                            bH = gb_sb[:, (l * 3 + 2) * KO + no:
                                       (l * 3 + 2) * KO + no + 1]
                            nc.scalar.activation(f_t[:], psF[:], AFT.Sigmoid,
                                                 bias=bF)
                            nc.scalar.activation(i_t[:], psI[:], AFT.Sigmoid,
                                                 bias=bI)
                            nc.vector.tensor_add(d_t[:], f_t[:], i_t[:])
                            nc.vector.reciprocal_approx_fast(r_t[:], d_t[:])
                            nc.vector.tensor_mul(g_t[:], i_t[:], r_t[:])
                            # A = 1 - g (single-src DVE op runs in 4x mode)
                            nc.vector.tensor_scalar(
                                a_t[:], g_t[:], -1.0, 1.0,
                                op0=OP.mult, op1=OP.add)
                            # B = (psH + bH) * g
                            nc.vector.scalar_tensor_tensor(
                                b_t[:], psH[:], bH, g_t[:],
                                op0=OP.add, op1=OP.mult)
                            base = no * BS + b * S
                            if half == 0:
                                nc.vector.tensor_tensor_scan(
                                    h_out[:, base: base + 512],
                                    a_t[:], b_t[:],
                                    initial=0.0, op0=OP.mult, op1=OP.add)
                            else:
                                nc.vector.tensor_tensor_scan(
                                    h_out[:, base + 512: base + S],
                                    a_t[:], b_t[:],
                                    initial=h_out[:, base + 511: base + 512],
                                    op0=OP.mult, op1=OP.add)
                    X = h_out
                    if l + 1 < L:
                        gws = gws_next

            gw_pool.__exit__(None, None, None)
            h4 = X

            # ---------------- attention (last query position only) ----------
            with (
                tc.tile_pool(name="vp", bufs=1) as vp,
                tc.tile_pool(name="owp", bufs=1) as owp,
                tc.tile_pool(name="smallp", bufs=1) as smallp,
            ):
                ow_sb = owp.tile([P, KO * H], DT)
                nc.sync.dma_start(ow_sb[:], ow[:])
                fcw_sb = owp.tile([P, KO * O], DT)
                nc.sync.dma_start(fcw_sb[:], fcw[:])
                ipb_sb = constp.tile([P, 2 * KO], F32)
                nc.sync.dma_start(ipb_sb[:], ipb[:])
                vb_sb = constp.tile([P, NH], F32)
                nc.sync.dma_start(vb_sb[:], vb[:])
                ob_sb = constp.tile([P, KO], F32)
                nc.sync.dma_start(ob_sb[:], ob[:])
                fcb_sb = constp.tile([P, O // P], F32)
                nc.sync.dma_start(fcb_sb[:], fcb[:])

                V_sb = vp.tile([P, KO * BS], DT, name="V_sb")
                ipk2_sb = vp.tile([P, NH * H], DT, name="ipk2_sb")
                nc.sync.dma_start(ipk2_sb[:], ipk2[:])
                lastq = smallp.tile([P, 2 * KO], DT)
                q_sb = smallp.tile([P, 2 * KO], DT)
                qt_sb = smallp.tile([P, KO * BC * NH], DT)   # [128, 128]
                e_all = smallp.tile([P, BC * KO * NH], DT)   # [128, 128]
                den_r = smallp.tile([1, BC * NH], F32)
                rb_sb = smallp.tile([P, BC * NH], F32)
                O_last = smallp.tile([P, 2 * KO], DT)
                out_last = smallp.tile([P, 2 * KO], DT)
                res_sb = smallp.tile([P, 2 * (O // P)], F32)

                # h4 columns at the last timestep (per ko-tile, per batch)
                for ko in range(KO):
                    for b in range(BC):
                        nc.vector.tensor_copy(
                            lastq[:, ko * BC + b: ko * BC + b + 1],
                            h4[:, ko * BS + b * S + S - 1:
                               ko * BS + b * S + S])

                with (
                    tc.tile_pool(name="psT", bufs=1, space="PSUM") as psT,
                ):
                    def emit_V(b):
                        # V (position-major) for batch b
                        for si in range(KO):
                            for dch in range(2):
                                d0 = dch * 512
                                psv = psV.tile([P, 512], F32, tag="v",
                                               name="psv", bufs=2)
                                for ko in range(KO):
                                    nc.tensor.matmul(
                                        psv[:],
                                        h4[:, ko * BS + b * S + si * P:
                                           ko * BS + b * S + (si + 1) * P],
                                        ip_sb[:, ko * 2 * H + H + d0:
                                              ko * 2 * H + H + d0 + 512],
                                        start=(ko == 0), stop=(ko == KO - 1))
                                st = b * KO + si
                                nc.scalar.activation(
                                    V_sb[:, st * H + d0: st * H + d0 + 512],
                                    psv[:], AFT.Copy)

                    def emit_eV(b):
                        for j in range(NH):
                            ps_o_t = psT.tile([P, BC], F32, tag="sm2",
                                              name="ps_o", bufs=2)
                            ps_o = ps_o_t[:, 0:1]
                            for kt in range(KO):
                                nc.tensor.matmul(
                                    ps_o,
                                    V_sb[:, (b * KO + kt) * H + j * P:
                                         (b * KO + kt) * H + (j + 1) * P],
                                    e_all[:, (b * KO + kt) * NH + j:
                                          (b * KO + kt) * NH + j + 1],
                                    start=(kt == 0), stop=(kt == KO - 1))
                            nc.vector.scalar_tensor_tensor(
                                O_last[:, j * BC + b: j * BC + b + 1],
                                ps_o, rb_sb[:, b * NH + j: b * NH + j + 1],
                                vb_sb[:, j: j + 1],
                                op0=OP.mult, op1=OP.add)

                    # V(b=0) first: fills the PE while the last layer's scan
                    # chain and lastq drain
                    emit_V(0)
                    # q at the last position (head j occupies d-chunk j)
                    for j in range(NH):
                        psq = psT.tile([P, BC], F32, tag="sm2", name="psq",
                                       bufs=2)
                        for ko in range(KO):
                            nc.tensor.matmul(
                                psq[:],
                                ip_sb[:, ko * 2 * H + j * P:
                                      ko * 2 * H + (j + 1) * P],
                                lastq[:, ko * BC: (ko + 1) * BC],
                                start=(ko == 0), stop=(ko == KO - 1))
                        nc.scalar.activation(
                            q_sb[:, j * BC: (j + 1) * BC], psq[:],
                            AFT.Identity, bias=ipb_sb[:, j: j + 1],
                            scale=QSCALE)
                    # q~_j = Wk_j^T q_j  (folded-K scores vector)
                    qt_v = qt_sb.rearrange("p (hc b j) -> p hc b j",
                                           hc=KO, b=BC)
                    for j in range(NH):
                        psqt = psT.tile([P, KO * BC], F32, tag="w16",
                                        name="psqt", bufs=3)
                        for hc in range(KO):
                            nc.tensor.matmul(
                                psqt[:, hc * BC: (hc + 1) * BC],
                                ipk2_sb[:, j * H + hc * P:
                                        j * H + (hc + 1) * P],
                                q_sb[:, j * BC: (j + 1) * BC],
                                start=True, stop=True)
                        psqt_v = psqt.rearrange("p (hc b) -> p hc b", hc=KO)
                        nc.scalar.activation(
                            qt_v[:, :, :, j], psqt_v[:, :, :], AFT.Copy)
                    # scores via q~ . h4 (s on partitions) + exp + denom
                    ps_den_t = psT.tile([P, BC * NH], F32, tag="den",
                                        name="ps_den")
                    ps_den = ps_den_t[0:1, :]
                    for b in range(BC):
                        for kt in range(KO):
                            pss_t = psT.tile([P, KO * BC], F32, tag="w16",
                                              name="pss", bufs=3)
                            pss = pss_t[:, :NH]
                            for ko in range(KO):
                                nc.tensor.matmul(
                                    pss,
                                    h4[:, ko * BS + b * S + kt * P:
                                       ko * BS + b * S + (kt + 1) * P],
                                    qt_sb[:, ko * BC * NH + b * NH:
                                          ko * BC * NH + (b + 1) * NH],
                                    start=(ko == 0), stop=(ko == KO - 1))
                            eix = (b * KO + kt) * NH
                            nc.scalar.activation(
                                e_all[:, eix: eix + NH], pss, AFT.Exp)
                            nc.tensor.matmul(
                                ps_den[:, b * NH: (b + 1) * NH],
                                ones_col[:],
                                e_all[:, eix: eix + NH],
                                start=(kt == 0), stop=(kt == KO - 1))
                    nc.vector.reciprocal(den_r[:], ps_den)
                    # broadcast reciprocal across partitions -> [128, 16]
                    ps_bc = psT.tile([P, BC * NH], F32, tag="w16", name="ps_bc", bufs=3)
                    nc.tensor.matmul(ps_bc[:], ones_row[:], den_r[:],
                                     start=True, stop=True)
                    nc.scalar.activation(rb_sb[:], ps_bc[:], AFT.Copy)
                    # eV(b0) before V(b1): its deps (V(b0) drains + rb) are
                    # ready, and it shortens the post-V(b1) critical tail
                    emit_eV(0)
                    emit_V(1)
                    emit_eV(1)
                    # out projection at last position + residual
                    for no in range(KO):
                        ps_p = psT.tile([P, BC], F32, tag="sm2", name="ps_p",
                                        bufs=2)
                        for ko in range(KO):
                            nc.tensor.matmul(
                                ps_p[:],
                                ow_sb[:, ko * H + no * P: ko * H + (no + 1) * P],
                                O_last[:, ko * BC: (ko + 1) * BC],
                                start=(ko == 0), stop=(ko == KO - 1))
                        nc.vector.scalar_tensor_tensor(
                            out_last[:, no * BC: (no + 1) * BC],
                            ps_p[:], ob_sb[:, no:no + 1],
                            lastq[:, no * BC: (no + 1) * BC],
                            op0=OP.add, op1=OP.add)
                    # final fc
                    for ot in range(O // P):
                        ps_f = psT.tile([P, BC], F32, tag="sm2", name="ps_f",
                                        bufs=2)
                        for ko in range(KO):
                            nc.tensor.matmul(
                                ps_f[:],
                                fcw_sb[:, ko * O + ot * P: ko * O + (ot + 1) * P],
                                out_last[:, ko * BC: (ko + 1) * BC],
                                start=(ko == 0), stop=(ko == KO - 1))
                        nc.scalar.activation(
                            res_sb[:, ot * BC: (ot + 1) * BC], ps_f[:],
                            AFT.Identity, bias=fcb_sb[:, ot:ot + 1])
                        nc.sync.dma_start(
                            outT[ot * P:(ot + 1) * P, :],
                            res_sb[:, ot * BC: (ot + 1) * BC])

            psV_pool.__exit__(None, None, None)
            ip_pool.__exit__(None, None, None)

    nc.compile()
    return nc


def _feature_major(w_t):
    """[H_in, N] (already transposed weight) -> device layout [128, KO*N]."""
    hin, n = w_t.shape
    ko = hin // P
    return np.ascontiguousarray(
        w_t.reshape(ko, P, n).transpose(1, 0, 2).reshape(P, ko * n))


def _prep_inputs(x, Wf, bf, Wi, bi, Wh, bh, in_proj_w, in_proj_b, out_w,
                 out_b, fc_w, fc_b):
    gws = []
    gbs = []
    for l in range(L):
        for W, bias in ((Wf[l], bf[l]), (Wi[l], bi[l]), (Wh[l], bh[l])):
            gws.append(_feature_major(W.T.astype(np.float32)).astype(BF16))
            gbs.append(bias.reshape(KO, P).T.astype(np.float32))
    gw = np.concatenate(gws, axis=0)                     # [12*128, KO*H]
    gb = np.concatenate(gbs, axis=1)                     # [128, 12*KO]
    w_t = in_proj_w.T.astype(np.float32)                 # [h, 3H]
    qv = np.concatenate([w_t[:, :H], w_t[:, 2 * H:]], axis=1)  # [h, 2H]
    ip = _feature_major(qv).astype(BF16)
    # Wk laid out d-major for the folded-K trick: [128 (dd), j*H + h]
    wk = in_proj_w[H:2 * H].astype(np.float32)           # [d, h]
    ipk2 = np.ascontiguousarray(
        wk.reshape(NH, DH, H).transpose(1, 0, 2).reshape(DH, NH * H)
    ).astype(BF16)
    ipb = in_proj_b[:2 * H].reshape(2 * KO, P).T.astype(np.float32).copy()
    ipb[:, :KO] *= QSCALE                                # fold Q scaling
    vbv = in_proj_b[2 * H:].reshape(NH, P).T.astype(np.float32)
    owp = _feature_major(out_w.T.astype(np.float32)).astype(BF16)
    obv = out_b.reshape(KO, P).T.astype(np.float32)
    fcwp = _feature_major(fc_w.T.astype(np.float32)).astype(BF16)
    fcbv = fc_b.reshape(O // P, P).T.astype(np.float32)
    shared = dict(gw=gw, gb=np.ascontiguousarray(gb),
                  ip=ip, ipk2=ipk2, ipb=np.ascontiguousarray(ipb),
                  vb=np.ascontiguousarray(vbv), ow=owp,
                  ob=np.ascontiguousarray(obv), fcw=fcwp,
                  fcb=np.ascontiguousarray(fcbv))
    in_maps = []
    for c in range(NCORES):
        shard = x[c * BC:(c + 1) * BC]                   # [BC, S, H]
        xt = shard.transpose(2, 0, 1).reshape(H, BS)     # [H, BS]
        xt = _feature_major(xt)                          # [128, KO*BS]
        # chunk-major [128, ch(4), ko(8), 512] so chunk DMAs are contiguous
        xt = np.ascontiguousarray(
            xt.reshape(P, KO, 4, 512).transpose(0, 2, 1, 3).reshape(P, KO * BS)
        ).astype(BF16)
        in_maps.append(dict(shared, xT=xt))
    return in_maps


def kernel(x, Wf, bf, Wi, bi, Wh, bh, in_proj_w, in_proj_b, out_w, out_b,
           fc_w, fc_b):
    from concourse.bass_utils import run_bass_kernel_spmd

    x, Wf, bf, Wi, bi, Wh, bh = (np.asarray(t) for t in
                                 (x, Wf, bf, Wi, bi, Wh, bh))
    in_proj_w, in_proj_b, out_w, out_b, fc_w, fc_b = (
        np.asarray(t) for t in (in_proj_w, in_proj_b, out_w, out_b,
                                fc_w, fc_b))
    if "nc" not in _CACHE:
        _CACHE["nc"] = _build_nc()
    nc = _CACHE["nc"]
    in_maps = _prep_inputs(x, Wf, bf, Wi, bi, Wh, bh, in_proj_w, in_proj_b,
                           out_w, out_b, fc_w, fc_b)
    res = run_bass_kernel_spmd(nc, in_maps, core_ids=list(range(NCORES)))
    _CACHE["last_results"] = res
    out = np.empty((B, O), np.float32)
    for c in range(NCORES):
        outT = res.results[c]["outT"]                    # [O, BC]
        for b in range(BC):
            out[c * BC + b] = outT[:, b]
    return out
